# revision 9
# baseline (speedup 1.0000x reference)
"""Compact Bilinear Pooling on 8 Trainium2 NeuronCores (Bass/Tile).

Math: the reference computes, per batch image b,
    out[b] = sum_{pixels n} irfft( rfft(b1[n] @ S1) * rfft(b2[n] @ S2) )
Since irfft is linear and the sum-pool happens after it, this collapses to
    out[b] = irfft( sum_n rfft(sketch1[n]) * rfft(sketch2[n]) ).
And rfft(x @ S) = x @ F where F[c, k] = s[c] * exp(-2i*pi*h[c]*k/D) is a dense
"DFT of the count-sketch" matrix, precomputable on the host from S (each row of
S has a single nonzero s[c] at column h[c]).  So per batch image:
  stage 1: fftX[k, n] = sum_c F[c, k] * bX[c, n]      (PE matmuls, bf16)
  stage 2: Spec[k]    = sum_n fft1[k, n]*fft2[k, n]   (fused DVE mult+reduce,
                                                       fft2 staged via ScalarE)
  stage 4: out[b]     = irfft(Spec)                   (tiny Cooley-Tukey 128x64
                                                       via fp32 PE matmuls)
Conjugate symmetry: only k = 0..4096 computed (padded to 33 tiles of 128).
DC/Nyquist 1/2-weights and the zeroing of pad bins are folded into F1 on host.
Sharding: data-parallel over batch, 4 images per core, no cross-core comm.
"""

import numpy as np
import ml_dtypes

import concourse.bacc as bacc
import concourse.mybir as mybir
import concourse.tile as tile
from concourse.bass_utils import run_bass_kernel_spmd
from concourse.masks import make_identity

# problem shapes (hardcoded per contract)
B, C, HH, WW = 32, 512, 14, 14
HW = HH * WW            # 196 pixels
HW2 = 2 * HW            # a batch-pair of pixels
D = 8192                # sketch/output dim
NF = D // 2 + 1         # 4097 rfft bins
NMT = 33                # freq tiles of 128 (33*128 = 4224 >= 4097)
FPAD = NMT * 128        # 4224
NCORES = 8
NB = B // NCORES        # 4 batch images per core
KC = C // 128           # 4 contraction chunks
P = 128
F32 = mybir.dt.float32
BF16 = mybir.dt.bfloat16
BF16_NP = ml_dtypes.bfloat16

TRACE = False           # set by test harness for profiling runs
LAST_RESULTS = None     # BassKernelResults of the last run (for the harness)

_CACHE = {}


# ---------------------------------------------------------------- host consts
def _extract_sketch(S):
    """S is [C, D] with one nonzero (+-1) per row -> (h, s)."""
    S = np.asarray(S, dtype=np.float32)
    h = np.argmax(np.abs(S), axis=1)
    s = S[np.arange(S.shape[0]), h]
    return h, s


def _make_F(S, half_edges):
    """F[c, k] = s[c]*exp(-2i pi h[c] k / D), k in [0, FPAD); pad bins zeroed.
    half_edges also folds the irfft 1/2 weight of the DC/Nyquist bins in.
    Returned layout: [128, KC, FPAD] bf16 (partition-major for one-shot DMA)."""
    h, s = _extract_sketch(S)
    k = np.arange(FPAD)
    ang = (2.0 * np.pi / D) * np.outer(h.astype(np.float64), k)
    Fr = s[:, None] * np.cos(ang)
    Fi = -s[:, None] * np.sin(ang)
    Fr[:, NF:] = 0.0
    Fi[:, NF:] = 0.0
    if half_edges:
        Fr[:, 0] *= 0.5
        Fr[:, D // 2] *= 0.5
        Fi[:, 0] *= 0.5
        Fi[:, D // 2] *= 0.5
    Fr = np.ascontiguousarray(Fr.reshape(KC, 128, FPAD).transpose(1, 0, 2))
    Fi = np.ascontiguousarray(Fi.reshape(KC, 128, FPAD).transpose(1, 0, 2))
    return Fr.astype(BF16_NP), Fi.astype(BF16_NP)


# column offsets inside the packed [128, 576] fp32 constant blob
_CB = {"e32r": 0, "e32i": 64, "e32ni": 128, "twr": 192, "twi": 256,
       "w128r": 320, "w128ni": 448}
_CBW = 576


def _ifft_consts():
    """irfft(Spec)[64*j1 + j2] = 2/D * Re( sum_k1 W[k1,j1] T[k1,j2]
                                   * sum_k2 Spec[k1 + 128*k2] E[k2,j2] ).
    All constants packed into one [128, 576] fp32 blob."""
    blob = np.zeros((P, _CBW), dtype=np.float32)
    j2 = np.arange(64)[None, :]
    k2 = np.arange(NMT)[:, None]
    angE = 2.0 * np.pi * k2 * j2 / 64.0
    blob[:NMT, 0:64] = (2.0 / D) * np.cos(angE)
    blob[:NMT, 64:128] = (2.0 / D) * np.sin(angE)
    blob[:NMT, 128:192] = -(2.0 / D) * np.sin(angE)
    k1 = np.arange(P)[:, None]
    angT = 2.0 * np.pi * k1 * j2 / D
    blob[:, 192:256] = np.cos(angT)
    blob[:, 256:320] = np.sin(angT)
    j1 = np.arange(P)[None, :]
    angW = 2.0 * np.pi * np.arange(P)[:, None] * j1 / 128.0
    blob[:, 320:448] = np.cos(angW)
    blob[:, 448:576] = -np.sin(angW)
    return blob


def _shard_bottom(bottom):
    """[B, C, 14, 14] f32 -> per-core [128, KC, NB, 196] bf16."""
    a = np.asarray(bottom, dtype=np.float32).reshape(NCORES, NB, KC, 128, HW)
    a = np.ascontiguousarray(a.transpose(0, 3, 2, 1, 4)).astype(BF16_NP)
    return [np.ascontiguousarray(a[i]) for i in range(NCORES)]


# ---------------------------------------------------------------- bass program
def _build_nc():
    nc = bacc.Bacc("TRN2", target_bir_lowering=False, num_devices=NCORES)

    b1_d = nc.dram_tensor("b1", [P, KC, NB, HW], BF16, kind="ExternalInput")
    b2_d = nc.dram_tensor("b2", [P, KC, NB, HW], BF16, kind="ExternalInput")
    f_d = {w: nc.dram_tensor(w, [P, KC, FPAD], BF16, kind="ExternalInput")
           for w in ("f1r", "f1i", "f2r", "f2i")}
    cb_d = nc.dram_tensor("cblob", [P, _CBW], F32, kind="ExternalInput")
    out_d = nc.dram_tensor("out", [NB, D], F32, kind="ExternalOutput")
    out_v = out_d.ap().rearrange("b (p f) -> b p f", p=P)

    mult = mybir.AluOpType.mult

    with tile.TileContext(nc) as tc:
        with (
            tc.tile_pool(name="consts", bufs=1) as consts,
            tc.tile_pool(name="spec", bufs=1) as specp,
            tc.tile_pool(name="scratch", bufs=3) as scratch,
            tc.tile_pool(name="ps", bufs=1, space="PSUM") as psp,
        ):
            # --- weights: first freq chunk lands before anything else so the
            # PE can start early; remaining chunks stream in freq-major order.
            fw = {w: consts.tile([P, KC, FPAD], BF16, name=f"fw_{w}", tag=f"fw_{w}")
                  for w in f_d}
            CH0 = 512
            for w in fw:
                nc.sync.dma_start(out=fw[w][:, :, 0:CH0],
                                  in_=f_d[w].ap()[:, :, 0:CH0])
            # bottom activations + ifft consts (issued via gpsimd queue so they
            # don't serialize behind the weight chunks on the sync queue)
            bt1 = consts.tile([P, KC, NB, HW], BF16, name="bt1", tag="bt1")
            bt2 = consts.tile([P, KC, NB, HW], BF16, name="bt2", tag="bt2")
            nc.gpsimd.dma_start(out=bt1, in_=b1_d.ap())
            nc.gpsimd.dma_start(out=bt2, in_=b2_d.ap())
            cblob = consts.tile([P, _CBW], F32, name="cblob", tag="cblob")
            nc.gpsimd.dma_start(out=cblob, in_=cb_d.ap())
            identity = consts.tile([P, P], F32, name="identity", tag="identity")
            make_identity(nc, identity)
            CHUNK = 1024
            for g0 in range(CH0, FPAD, CHUNK):
                g1 = min(g0 + CHUNK, FPAD)
                for w in fw:
                    nc.sync.dma_start(out=fw[w][:, :, g0:g1],
                                      in_=f_d[w].ap()[:, :, g0:g1])

            def cn(nm, parts=P):
                c0 = _CB[nm]
                wid = 128 if nm.startswith("w128") else 64
                return cblob[:parts, c0:c0 + wid]

            # --- spectra accumulators [128 = k mod 128, NMT = k div 128]
            specR = [specp.tile([P, NMT], F32, name=f"specR{b}", tag=f"specR{b}")
                     for b in range(NB)]
            specI = [specp.tile([P, NMT], F32, name=f"specI{b}", tag=f"specI{b}")
                     for b in range(NB)]

            # ---------------- stage 1+2 unit: one (freq-tile, batch-pair)
            def unit(mt, bp):
                c0 = mt * P
                bsl2 = slice(bp * HW2, (bp + 1) * HW2)
                b1f = [bt1[:, kc].rearrange("p b h -> p (b h)")[:, bsl2]
                       for kc in range(KC)]
                b2f = [bt2[:, kc].rearrange("p b h -> p (b h)")[:, bsl2]
                       for kc in range(KC)]
                # p2 first: its single rotation slot is freed by the ScalarE
                # copies while this unit's p1 matmuls still run.
                p2 = psp.tile([P, 2, 512], F32, name=f"p2_{mt}_{bp}", tag="p2",
                              bufs=1)
                p1 = psp.tile([P, 2, 512], F32, name=f"p1_{mt}_{bp}", tag="p1",
                              bufs=2)
                for reg, wn in ((0, "f2r"), (1, "f2i")):
                    for kc in range(KC):
                        nc.tensor.matmul(p2[:, reg, 0:HW2],
                                         fw[wn][:, kc, c0:c0 + P], b2f[kc],
                                         start=kc == 0, stop=kc == KC - 1)
                for reg, wn in ((0, "f1r"), (1, "f1i")):
                    for kc in range(KC):
                        nc.tensor.matmul(p1[:, reg, 0:HW2],
                                         fw[wn][:, kc, c0:c0 + P], b1f[kc],
                                         start=kc == 0, stop=kc == KC - 1)
                # fft2 staged through SBUF with product signs/swaps folded in
                # via overlapping windows: sbBIG = [f2i | f2r | -f2i];
                # rows 0:2 feed SpecI, rows 1:3 feed SpecR.
                sbBIG = scratch.tile([P, 3, HW2], F32, name=f"sbBIG_{mt}_{bp}",
                                     tag="sbBIG")
                nc.scalar.copy(sbBIG[:, 0, :], p2[:, 1, 0:HW2])
                nc.scalar.copy(sbBIG[:, 1, :], p2[:, 0, 0:HW2])
                nc.scalar.mul(sbBIG[:, 2, :], p2[:, 1, 0:HW2], -1.0)
                dst = scratch.tile([P, 2, HW], F32, name=f"dst_{mt}_{bp}",
                                   tag="dst")
                dst2 = scratch.tile([P, 2, HW], F32, name=f"dst2_{mt}_{bp}",
                                    tag="dst2")
                for j in range(2):
                    b = 2 * bp + j
                    bsl = slice(j * HW, (j + 1) * HW)
                    nc.vector.scalar_tensor_tensor(
                        out=dst, in0=p1[:, :, bsl], scalar=1.0,
                        in1=sbBIG[:, 1:3, bsl], op0=mult, op1=mult,
                        accum_out=specR[b][:, mt:mt + 1])
                    nc.vector.scalar_tensor_tensor(
                        out=dst2, in0=p1[:, :, bsl], scalar=1.0,
                        in1=sbBIG[:, 0:2, bsl], op0=mult, op1=mult,
                        accum_out=specI[b][:, mt:mt + 1])

            # ---------------- stage 3+4 for one image, as 5 pieces that get
            # interleaved under the next batch-pair's stage-1 work.
            def stage4_pieces(b):
                st = {}

                def pc_tr_r():
                    st["ptr"] = psp.tile([NMT, P], F32, name=f"ptrR_{b}",
                                         tag="tr", bufs=1)
                    nc.tensor.transpose(st["ptr"], specR[b], identity)
                    st["s2r"] = scratch.tile([NMT, P], F32, name=f"s2r_{b}",
                                             tag="s2r")
                    nc.scalar.copy(st["s2r"], st["ptr"])

                def pc_tr_i():
                    st["pti"] = psp.tile([NMT, P], F32, name=f"ptrI_{b}",
                                         tag="tr", bufs=1)
                    nc.tensor.transpose(st["pti"], specI[b], identity)
                    st["s2i"] = scratch.tile([NMT, P], F32, name=f"s2i_{b}",
                                             tag="s2i")
                    nc.scalar.copy(st["s2i"], st["pti"])

                def pc_u():
                    # uu bank: region 0 = Ur, 1 = Ui, 2 = X (later)
                    uu = st["uu"] = psp.tile([P, 3, 64], F32, name=f"uu_{b}",
                                             tag="uu", bufs=1)
                    nc.tensor.matmul(uu[:, 0, :], st["s2r"], cn("e32r", NMT),
                                     start=True, stop=False)
                    nc.tensor.matmul(uu[:, 0, :], st["s2i"], cn("e32ni", NMT),
                                     start=False, stop=True)
                    nc.tensor.matmul(uu[:, 1, :], st["s2r"], cn("e32i", NMT),
                                     start=True, stop=False)
                    nc.tensor.matmul(uu[:, 1, :], st["s2i"], cn("e32r", NMT),
                                     start=False, stop=True)

                def pc_tw():
                    uu = st["uu"]
                    vr = st["vr"] = scratch.tile([P, 64], F32, name=f"vr_{b}",
                                                 tag="vr")
                    vi = st["vi"] = scratch.tile([P, 64], F32, name=f"vi_{b}",
                                                 tag="vi")
                    ta = scratch.tile([P, 64], F32, name=f"ta_{b}", tag="ta")
                    tb = scratch.tile([P, 64], F32, name=f"tb_{b}", tag="tb")
                    nc.vector.tensor_mul(vr, uu[:, 0, :], cn("twr"))
                    nc.vector.tensor_mul(ta, uu[:, 1, :], cn("twi"))
                    nc.vector.tensor_sub(vr, vr, ta)
                    nc.vector.tensor_mul(vi, uu[:, 0, :], cn("twi"))
                    nc.vector.tensor_mul(tb, uu[:, 1, :], cn("twr"))
                    nc.vector.tensor_add(vi, vi, tb)

                def pc_x():
                    uu = st["uu"]
                    nc.tensor.matmul(uu[:, 2, :], cn("w128r"), st["vr"],
                                     start=True, stop=False)
                    nc.tensor.matmul(uu[:, 2, :], cn("w128ni"), st["vi"],
                                     start=False, stop=True)
                    xo = scratch.tile([P, 64], F32, name=f"xo_{b}", tag="xo")
                    nc.vector.tensor_copy(out=xo, in_=uu[:, 2, :])
                    nc.sync.dma_start(out=out_v[b], in_=xo)

                return [pc_tr_r, pc_tr_i, pc_u, pc_tw, pc_x]

            # ---------------- main loop: bp outer so each pair's stage-4 can
            # hide under the next pair's stage-1.
            pending = []
            for bp in range(NB // 2):
                for mt in range(NMT):
                    unit(mt, bp)
                    if pending:
                        pending.pop(0)()
                pending += stage4_pieces(2 * bp)
                pending += stage4_pieces(2 * bp + 1)
            for pc in pending:
                pc()

    nc.compile()
    return nc


# ---------------------------------------------------------------- entry point
def kernel(bottom1, bottom2, S1, S2):
    global LAST_RESULTS
    bottom1 = np.asarray(bottom1, dtype=np.float32)
    bottom2 = np.asarray(bottom2, dtype=np.float32)

    if "nc" not in _CACHE:
        _CACHE["nc"] = _build_nc()
    nc = _CACHE["nc"]

    f1r, f1i = _make_F(S1, half_edges=True)
    f2r, f2i = _make_F(S2, half_edges=False)
    shared = {"f1r": f1r, "f1i": f1i, "f2r": f2r, "f2i": f2i,
              "cblob": _ifft_consts()}

    b1s = _shard_bottom(bottom1)
    b2s = _shard_bottom(bottom2)
    in_maps = [{"b1": b1s[i], "b2": b2s[i], **shared} for i in range(NCORES)]

    res = run_bass_kernel_spmd(nc, in_maps, core_ids=list(range(NCORES)),
                               trace=TRACE)
    LAST_RESULTS = res
    out = np.concatenate([r["out"] for r in res.results], axis=0)
    return out.astype(np.float32)


# revision 10
# speedup vs baseline: 1.2138x; 1.2138x over previous
"""Compact Bilinear Pooling on 8 Trainium2 NeuronCores (Bass/Tile).

Math: the reference computes, per batch image b,
    out[b] = sum_{pixels n} irfft( rfft(b1[n] @ S1) * rfft(b2[n] @ S2) )
Since irfft is linear and the sum-pool happens after it, this collapses to
    out[b] = irfft( sum_n rfft(sketch1[n]) * rfft(sketch2[n]) ).
And rfft(x @ S) = x @ F where F[c, k] = s[c] * exp(-2i*pi*h[c]*k/D) is a dense
"DFT of the count-sketch" matrix, precomputable on the host from S (each row of
S has a single nonzero s[c] at column h[c]).  So per batch image:
  stage 1: fftX[k, n] = sum_c F[c, k] * bX[c, n]      (PE matmuls, bf16)
  stage 2: Spec[k]    = sum_n fft1[k, n]*fft2[k, n]   (fused DVE mult+reduce,
                                                       fft2 staged via ScalarE)
  stage 4: out[b]     = irfft(Spec)                   (tiny Cooley-Tukey 128x64
                                                       via fp32 PE matmuls)
Conjugate symmetry: only k = 0..4096 computed (padded to 33 tiles of 128).
DC/Nyquist 1/2-weights and the zeroing of pad bins are folded into F1 on host.
Sharding: data-parallel over batch, 4 images per core, no cross-core comm.
"""

import numpy as np
import ml_dtypes

import concourse.bacc as bacc
import concourse.mybir as mybir
import concourse.tile as tile
from concourse.bass_utils import run_bass_kernel_spmd
from concourse.masks import make_identity

# problem shapes (hardcoded per contract)
B, C, HH, WW = 32, 512, 14, 14
HW = HH * WW            # 196 pixels
HW2 = 2 * HW            # a batch-pair of pixels
D = 8192                # sketch/output dim
NF = D // 2 + 1         # 4097 rfft bins
NMT = 33                # freq tiles of 128 (33*128 = 4224 >= 4097)
FPAD = NMT * 128        # 4224
NCORES = 8
NB = B // NCORES        # 4 batch images per core
KC = C // 128           # 4 contraction chunks
P = 128
F32 = mybir.dt.float32
BF16 = mybir.dt.bfloat16
BF16_NP = ml_dtypes.bfloat16

TRACE = False           # set by test harness for profiling runs
LAST_RESULTS = None     # BassKernelResults of the last run (for the harness)

_CACHE = {}


# ---------------------------------------------------------------- host consts
def _extract_sketch(S):
    """S is [C, D] with one nonzero (+-1) per row -> (h, s)."""
    S = np.asarray(S, dtype=np.float32)
    h = np.argmax(np.abs(S), axis=1)
    s = S[np.arange(S.shape[0]), h]
    return h, s


def _make_F(S, half_edges):
    """F[c, k] = s[c]*exp(-2i pi h[c] k / D), k in [0, FPAD); pad bins zeroed.
    half_edges also folds the irfft 1/2 weight of the DC/Nyquist bins in.
    Returned layout: [128, KC, FPAD] bf16 (partition-major for one-shot DMA)."""
    h, s = _extract_sketch(S)
    k = np.arange(FPAD)
    ang = (2.0 * np.pi / D) * np.outer(h.astype(np.float64), k)
    Fr = s[:, None] * np.cos(ang)
    Fi = -s[:, None] * np.sin(ang)
    Fr[:, NF:] = 0.0
    Fi[:, NF:] = 0.0
    if half_edges:
        Fr[:, 0] *= 0.5
        Fr[:, D // 2] *= 0.5
        Fi[:, 0] *= 0.5
        Fi[:, D // 2] *= 0.5
    Fr = np.ascontiguousarray(Fr.reshape(KC, 128, FPAD).transpose(1, 0, 2))
    Fi = np.ascontiguousarray(Fi.reshape(KC, 128, FPAD).transpose(1, 0, 2))
    return Fr.astype(BF16_NP), Fi.astype(BF16_NP)


# column offsets inside the packed [128, 576] fp32 constant blob
_CB = {"e32r": 0, "e32i": 64, "e32ni": 128, "twr": 192, "twi": 256,
       "w128r": 320, "w128ni": 448}
_CBW = 576


def _ifft_consts():
    """irfft(Spec)[64*j1 + j2] = 2/D * Re( sum_k1 W[k1,j1] T[k1,j2]
                                   * sum_k2 Spec[k1 + 128*k2] E[k2,j2] ).
    All constants packed into one [128, 576] fp32 blob."""
    blob = np.zeros((P, _CBW), dtype=np.float32)
    j2 = np.arange(64)[None, :]
    k2 = np.arange(NMT)[:, None]
    angE = 2.0 * np.pi * k2 * j2 / 64.0
    blob[:NMT, 0:64] = (2.0 / D) * np.cos(angE)
    blob[:NMT, 64:128] = (2.0 / D) * np.sin(angE)
    blob[:NMT, 128:192] = -(2.0 / D) * np.sin(angE)
    k1 = np.arange(P)[:, None]
    angT = 2.0 * np.pi * k1 * j2 / D
    blob[:, 192:256] = np.cos(angT)
    blob[:, 256:320] = np.sin(angT)
    j1 = np.arange(P)[None, :]
    angW = 2.0 * np.pi * np.arange(P)[:, None] * j1 / 128.0
    blob[:, 320:448] = np.cos(angW)
    blob[:, 448:576] = -np.sin(angW)
    return blob


def _shard_bottom(bottom):
    """[B, C, 14, 14] f32 -> per-core [128, KC, NB, 196] bf16."""
    a = np.asarray(bottom, dtype=np.float32).reshape(NCORES, NB, KC, 128, HW)
    a = np.ascontiguousarray(a.transpose(0, 3, 2, 1, 4)).astype(BF16_NP)
    return [np.ascontiguousarray(a[i]) for i in range(NCORES)]


# ---------------------------------------------------------------- bass program
def _build_nc():
    nc = bacc.Bacc("TRN2", target_bir_lowering=False, num_devices=NCORES)

    b1_d = nc.dram_tensor("b1", [P, KC, NB, HW], BF16, kind="ExternalInput")
    b2_d = nc.dram_tensor("b2", [P, KC, NB, HW], BF16, kind="ExternalInput")
    f_d = {w: nc.dram_tensor(w, [P, KC, FPAD], BF16, kind="ExternalInput")
           for w in ("f1r", "f1i", "f2r", "f2i")}
    cb_d = nc.dram_tensor("cblob", [P, _CBW], F32, kind="ExternalInput")
    out_d = nc.dram_tensor("out", [NB, D], F32, kind="ExternalOutput")
    out_v = out_d.ap().rearrange("b (p f) -> b p f", p=P)

    mult = mybir.AluOpType.mult

    with tile.TileContext(nc) as tc:
        with (
            tc.tile_pool(name="consts", bufs=1) as consts,
            tc.tile_pool(name="spec", bufs=1) as specp,
            tc.tile_pool(name="scratch", bufs=3) as scratch,
            tc.tile_pool(name="ps", bufs=1, space="PSUM") as psp,
        ):
            # --- weights: first freq chunk lands before anything else so the
            # PE can start early; remaining chunks stream in freq-major order.
            fw = {w: consts.tile([P, KC, FPAD], BF16, name=f"fw_{w}", tag=f"fw_{w}")
                  for w in f_d}
            CH0 = 512
            for w in fw:
                nc.sync.dma_start(out=fw[w][:, :, 0:CH0],
                                  in_=f_d[w].ap()[:, :, 0:CH0])
            # bottom activations + ifft consts (issued via gpsimd queue so they
            # don't serialize behind the weight chunks on the sync queue)
            bt1 = consts.tile([P, KC, NB, HW], BF16, name="bt1", tag="bt1")
            bt2 = consts.tile([P, KC, NB, HW], BF16, name="bt2", tag="bt2")
            nc.gpsimd.dma_start(out=bt1, in_=b1_d.ap())
            nc.gpsimd.dma_start(out=bt2, in_=b2_d.ap())
            cblob = consts.tile([P, _CBW], F32, name="cblob", tag="cblob")
            nc.gpsimd.dma_start(out=cblob, in_=cb_d.ap())
            identity = consts.tile([P, P], F32, name="identity", tag="identity")
            make_identity(nc, identity)
            CHUNK = 1024
            for g0 in range(CH0, FPAD, CHUNK):
                g1 = min(g0 + CHUNK, FPAD)
                for w in fw:
                    nc.sync.dma_start(out=fw[w][:, :, g0:g1],
                                      in_=f_d[w].ap()[:, :, g0:g1])

            def cn(nm, parts=P):
                c0 = _CB[nm]
                wid = 128 if nm.startswith("w128") else 64
                return cblob[:parts, c0:c0 + wid]

            # --- spectra accumulators [128 = k mod 128, NMT = k div 128]
            specR = [specp.tile([P, NMT], F32, name=f"specR{b}", tag=f"specR{b}")
                     for b in range(NB)]
            specI = [specp.tile([P, NMT], F32, name=f"specI{b}", tag=f"specI{b}")
                     for b in range(NB)]

            # ---------------- stage 1+2 unit: one (freq-tile, batch-pair)
            def unit(mt, bp):
                c0 = mt * P
                bsl2 = slice(bp * HW2, (bp + 1) * HW2)
                b1f = [bt1[:, kc].rearrange("p b h -> p (b h)")[:, bsl2]
                       for kc in range(KC)]
                b2f = [bt2[:, kc].rearrange("p b h -> p (b h)")[:, bsl2]
                       for kc in range(KC)]
                # p2 first: its single rotation slot is freed by the ScalarE
                # copies while this unit's p1 matmuls still run.
                p2 = psp.tile([P, 2, 512], F32, name=f"p2_{mt}_{bp}", tag="p2",
                              bufs=2)
                p1 = psp.tile([P, 2, 512], F32, name=f"p1_{mt}_{bp}", tag="p1",
                              bufs=2)
                for reg, wn in ((0, "f2r"), (1, "f2i")):
                    for kc in range(KC):
                        nc.tensor.matmul(p2[:, reg, 0:HW2],
                                         fw[wn][:, kc, c0:c0 + P], b2f[kc],
                                         start=kc == 0, stop=kc == KC - 1)
                for reg, wn in ((0, "f1r"), (1, "f1i")):
                    for kc in range(KC):
                        nc.tensor.matmul(p1[:, reg, 0:HW2],
                                         fw[wn][:, kc, c0:c0 + P], b1f[kc],
                                         start=kc == 0, stop=kc == KC - 1)
                # fft2 staged through SBUF with product signs/swaps folded in
                # via overlapping windows: sbBIG = [f2i | f2r | -f2i];
                # rows 0:2 feed SpecI, rows 1:3 feed SpecR.
                sbBIG = scratch.tile([P, 3, HW2], F32, name=f"sbBIG_{mt}_{bp}",
                                     tag="sbBIG")
                nc.scalar.copy(sbBIG[:, 0, :], p2[:, 1, 0:HW2])
                nc.scalar.copy(sbBIG[:, 1, :], p2[:, 0, 0:HW2])
                nc.scalar.mul(sbBIG[:, 2, :], p2[:, 1, 0:HW2], -1.0)
                dst = scratch.tile([P, 2, HW], F32, name=f"dst_{mt}_{bp}",
                                   tag="dst")
                dst2 = scratch.tile([P, 2, HW], F32, name=f"dst2_{mt}_{bp}",
                                    tag="dst2")
                for j in range(2):
                    b = 2 * bp + j
                    bsl = slice(j * HW, (j + 1) * HW)
                    nc.vector.scalar_tensor_tensor(
                        out=dst, in0=p1[:, :, bsl], scalar=1.0,
                        in1=sbBIG[:, 1:3, bsl], op0=mult, op1=mult,
                        accum_out=specR[b][:, mt:mt + 1])
                    nc.vector.scalar_tensor_tensor(
                        out=dst2, in0=p1[:, :, bsl], scalar=1.0,
                        in1=sbBIG[:, 0:2, bsl], op0=mult, op1=mult,
                        accum_out=specI[b][:, mt:mt + 1])

            # ---------------- stage 3+4 for one image, as 5 pieces that get
            # interleaved under the next batch-pair's stage-1 work.
            def stage4_pieces(b):
                st = {}

                def pc_tr_r():
                    big = psp.tile([P, 2, 512], F32, name=f"ptrR_{b}",
                                   tag="p2", bufs=2)
                    st["ptr"] = big[0:NMT, 0, 0:P]
                    nc.tensor.transpose(st["ptr"], specR[b], identity)
                    st["s2r"] = scratch.tile([NMT, P], F32, name=f"s2r_{b}",
                                             tag="s2r")
                    nc.scalar.copy(st["s2r"], st["ptr"])

                def pc_tr_i():
                    big = psp.tile([P, 2, 512], F32, name=f"ptrI_{b}",
                                   tag="p2", bufs=2)
                    st["pti"] = big[0:NMT, 0, 0:P]
                    nc.tensor.transpose(st["pti"], specI[b], identity)
                    st["s2i"] = scratch.tile([NMT, P], F32, name=f"s2i_{b}",
                                             tag="s2i")
                    nc.scalar.copy(st["s2i"], st["pti"])

                def pc_u():
                    # uu: region 0 = Ur, 1 = Ui, 2 = X (later)
                    big = psp.tile([P, 2, 512], F32, name=f"uu_{b}",
                                   tag="p2", bufs=2)
                    uu = st["uu"] = big[:, 0, 0:192].rearrange(
                        "p (r f) -> p r f", r=3)
                    nc.tensor.matmul(uu[:, 0, :], st["s2r"], cn("e32r", NMT),
                                     start=True, stop=False)
                    nc.tensor.matmul(uu[:, 0, :], st["s2i"], cn("e32ni", NMT),
                                     start=False, stop=True)
                    nc.tensor.matmul(uu[:, 1, :], st["s2r"], cn("e32i", NMT),
                                     start=True, stop=False)
                    nc.tensor.matmul(uu[:, 1, :], st["s2i"], cn("e32r", NMT),
                                     start=False, stop=True)

                def pc_tw():
                    uu = st["uu"]
                    vr = st["vr"] = scratch.tile([P, 64], F32, name=f"vr_{b}",
                                                 tag="vr")
                    vi = st["vi"] = scratch.tile([P, 64], F32, name=f"vi_{b}",
                                                 tag="vi")
                    ta = scratch.tile([P, 64], F32, name=f"ta_{b}", tag="ta")
                    tb = scratch.tile([P, 64], F32, name=f"tb_{b}", tag="tb")
                    nc.vector.tensor_mul(vr, uu[:, 0, :], cn("twr"))
                    nc.vector.tensor_mul(ta, uu[:, 1, :], cn("twi"))
                    nc.vector.tensor_sub(vr, vr, ta)
                    nc.vector.tensor_mul(vi, uu[:, 0, :], cn("twi"))
                    nc.vector.tensor_mul(tb, uu[:, 1, :], cn("twr"))
                    nc.vector.tensor_add(vi, vi, tb)

                def pc_x():
                    uu = st["uu"]
                    nc.tensor.matmul(uu[:, 2, :], cn("w128r"), st["vr"],
                                     start=True, stop=False)
                    nc.tensor.matmul(uu[:, 2, :], cn("w128ni"), st["vi"],
                                     start=False, stop=True)
                    xo = scratch.tile([P, 64], F32, name=f"xo_{b}", tag="xo")
                    nc.vector.tensor_copy(out=xo, in_=uu[:, 2, :])
                    nc.sync.dma_start(out=out_v[b], in_=xo)

                return [pc_tr_r, pc_tr_i, pc_u, pc_tw, pc_x]

            # ---------------- main loop: bp outer so each pair's stage-4 can
            # hide under the next pair's stage-1.
            pending = []
            for bp in range(NB // 2):
                for mt in range(NMT):
                    unit(mt, bp)
                    if pending:
                        pending.pop(0)()
                pending += stage4_pieces(2 * bp)
                pending += stage4_pieces(2 * bp + 1)
            for pc in pending:
                pc()

    nc.compile()
    return nc


# ---------------------------------------------------------------- entry point
def kernel(bottom1, bottom2, S1, S2):
    global LAST_RESULTS
    bottom1 = np.asarray(bottom1, dtype=np.float32)
    bottom2 = np.asarray(bottom2, dtype=np.float32)

    if "nc" not in _CACHE:
        _CACHE["nc"] = _build_nc()
    nc = _CACHE["nc"]

    f1r, f1i = _make_F(S1, half_edges=True)
    f2r, f2i = _make_F(S2, half_edges=False)
    shared = {"f1r": f1r, "f1i": f1i, "f2r": f2r, "f2i": f2i,
              "cblob": _ifft_consts()}

    b1s = _shard_bottom(bottom1)
    b2s = _shard_bottom(bottom2)
    in_maps = [{"b1": b1s[i], "b2": b2s[i], **shared} for i in range(NCORES)]

    res = run_bass_kernel_spmd(nc, in_maps, core_ids=list(range(NCORES)),
                               trace=TRACE)
    LAST_RESULTS = res
    out = np.concatenate([r["out"] for r in res.results], axis=0)
    return out.astype(np.float32)
